# revision 2
# baseline (speedup 1.0000x reference)
"""Trainium2 Bass kernel for per-channel piecewise-linear spline evaluation.

Reference op (nn_BSplineLayer): for u [4096, 64, 256], per-channel sorted
knots[256, 64] and coefs[256, 64]:
    i   = clip(searchsorted(knots_m, x, left) - 1, 0, 62)
    t   = (x - k_i) / (k_{i+1} - k_i + 1e-6)
    out = c_i + t * (c_{i+1} - c_i)

Device algorithm (gather-free): for x in [0, 1) the spline evaluates exactly
as a sum of saturating ramps,
    out(x) = c_0 + sum_{b=0..62} D_b * clamp((x - k_b) / (h_b + eps), 0, 1)
with D_b = c_{b+1} - c_b, h_b = k_{b+1} - k_b: the clamps of bins below the
active one saturate to 1 and telescope to c_i, the active bin contributes
t*D_i, bins above contribute 0 -- so no per-element gather/searchsorted is
needed. Two device forms:
  clamp1: one fused custom-DVE instruction per bin (63/tile, exact, 4.42 ms)
  relu2:  expand clamps into relus at unit-spaced thresholds (y = 63x) and
          fuse TWO bins per instruction (32/tile, ~7e-5 rel err, 2.33 ms)
Per-channel weights ride as per-partition [P,1] scalars; channels live on
the partition axis (two halves of 128), points on the free axis. Sharding:
data-parallel over the leading batch axis across the 8 cores; the tiny
knots/coefs-derived table is replicated.
"""

import sys

from functools import lru_cache

import numpy as np

try:
    import concourse.bacc as bacc  # noqa: F401
except ModuleNotFoundError:
    for _p in ("/opt/trn_rl_repo", "/root/.axon_site/_ro/trn_rl_repo"):
        if _p not in sys.path:
            sys.path.insert(0, _p)
    import concourse.bacc as bacc
import concourse.tile as tile
from concourse import mybir
from concourse import dve_ops as _dve_ops_mod
from concourse.bass_utils import run_bass_kernel_spmd
from concourse.dve_ops import DveOp
from concourse.dve_spec import (
    AluOp,
    Bin,
    C0,
    C1,
    C2,
    One,
    Spec,
    Src0,
    Src1,
    _has_src1,
    lower,
    minn,
    relu,
)
from concourse.dve_uop import DveOpSpec

F32 = mybir.dt.float32

N_CORES = 8
M_CHANNELS = 256
N_KNOTS = 64
EPS = 1e-6

# Tiling: per core 4096/8 * 64 = 32768 points, 256 channels in 2 halves of 128.
POINTS_PER_CORE = 32768
NF = 8192  # points per tile (free dim)
MODE = "relu2"  # "stock" | "clamp1" | "relu2"


# --------------------------------------------------------------------------- #
# Custom DVE ops
# --------------------------------------------------------------------------- #


def _register_dve_op(name: str, spec: Spec) -> DveOp:
    """Register a custom DVE op in-process (idempotent)."""
    for op in _dve_ops_mod.OPS:
        if op.name == name:
            return op
    row = _dve_ops_mod._CUSTOM_DVE_ROW_BASE + len(_dve_ops_mod.OPS)
    assert row < 0x20, "custom-DVE opcode rows exhausted"
    _dve_ops_mod._SUB_OPCODE_FOR_NAME[name] = row
    shas = {}
    for ver in ("v3", "v4"):
        try:
            tmp = DveOpSpec(
                name=name, opcode=row, uops=lower(spec, ver=ver),
                rd1_en=_has_src1(spec),
            )
            shas[ver] = tmp.sha(ver)
        except Exception:
            pass
    op = DveOp(name, spec, subdim=False, uops_sha=shas)
    _dve_ops_mod.OPS.append(op)
    _dve_ops_mod.CUSTOM_DVE_SPECS[name] = spec
    return op


# acc' = acc + wi * relu(min(x - k, cap));  s0=k, s1=wi=D*inv [P,1], imm2=cap=h+eps
SPLINE_CLAMP1 = _register_dve_op(
    "SPLINE_CLAMP1_ANT",
    Spec(
        body=Src1 + C1 * relu(minn(Src0 - C0, C2)),
        reference=lambda in0, in1, s0, s1, imm2: (
            in1
            + s1 * np.maximum(np.minimum(in0.astype(np.float32) - s0, imm2), 0.0)
        ).astype(np.float32),
    ),
)

# acc = c0 + wi * relu(min(x, cap))  (first bin; knots[0] == 0)
# s0=c0 [P,1], s1=wi0 [P,1], imm2=cap0
SPLINE_CLAMP1_INIT = _register_dve_op(
    "SPLINE_CLAMP1_INIT_ANT",
    Spec(
        body=C0 + C1 * relu(minn(Src0, C2)),
        reference=lambda in0, in1, s0, s1, imm2: (
            s0 + s1 * np.maximum(np.minimum(in0.astype(np.float32), imm2), 0.0)
        ).astype(np.float32),
    ),
)

# acc' = acc + g0 * relu(y - beta) + g1 * relu(y - (beta + 1))
# s0=g0 [P,1], s1=g1 [P,1], imm2=beta  (y pre-scaled so knots are ~1 apart;
# beta + 1 is stream-invariant -> hoisted to a swap flop, costs no lane)
SPLINE_RELU2 = _register_dve_op(
    "SPLINE_RELU2_ANT",
    Spec(
        body=Src1
        + C0 * relu(Src0 - C2)
        + C1 * relu(Src0 - Bin(AluOp.ADD, C2, One)),
        reference=lambda in0, in1, s0, s1, imm2: (
            in1
            + s0 * np.maximum(in0.astype(np.float32) - imm2, 0.0)
            + s1 * np.maximum(in0.astype(np.float32) - (imm2 + 1.0), 0.0)
        ).astype(np.float32),
    ),
)


# --------------------------------------------------------------------------- #
# Bass module
# --------------------------------------------------------------------------- #


@lru_cache(maxsize=4)
def _build_module(mode: str, n_points: int, nf: int, kb_key: tuple, cap_key: tuple,
                  reps: int = 1):
    """Build + compile the per-core Bass module.

    Inputs (per core):
      u_t  [256, n_points] f32  channel-major points
      tabs [256, TABW]     f32  per-channel scalar table (see _make_tabs)
    Output:
      out_t [256, n_points] f32
    """
    kb = np.asarray(kb_key, dtype=np.float64)  # 64 shared knots
    cap = np.asarray(cap_key, dtype=np.float64)  # 63 shared h+eps

    nc = bacc.Bacc("TRN2", target_bir_lowering=False)
    u_t = nc.dram_tensor("u_t", (M_CHANNELS, n_points), F32, kind="ExternalInput")
    tabs = nc.dram_tensor("tabs", (M_CHANNELS, 256), F32, kind="ExternalInput")
    out_t = nc.dram_tensor("out_t", (M_CHANNELS, n_points), F32, kind="ExternalOutput")

    n_tiles = n_points // nf
    assert n_points % nf == 0

    with tile.TileContext(nc) as tc:
        with (
            tc.tile_pool(name="tabp", bufs=1) as tabp,
            tc.tile_pool(name="xp", bufs=2) as xp,
            tc.tile_pool(name="accp", bufs=2) as accp,
        ):
            tab_tiles = []
            for hf in range(2):
                tt = tabp.tile([128, 256], F32, tag=f"tab{hf}")
                nc.sync.dma_start(tt[:], tabs[hf * 128:(hf + 1) * 128, :])
                tab_tiles.append(tt)

            for _rep in range(reps):
              for pt in range(n_tiles):
                for hf in range(2):
                    tt = tab_tiles[hf]
                    xt = xp.tile([128, nf], F32)
                    acc = accp.tile([128, nf], F32)
                    nc.sync.dma_start(
                        xt[:], u_t[hf * 128:(hf + 1) * 128, pt * nf:(pt + 1) * nf]
                    )
                    # tabs columns: 0 = c0, 1 + b = D_b*inv_b (b = 0..62)
                    if mode == "stock":
                        t = accp.tile([128, nf], F32, tag="tmp")
                        nc.vector.tensor_scalar(
                            acc[:], xt[:], 0.0, tt[:, 0:1],
                            mybir.AluOpType.mult, mybir.AluOpType.add,
                        )
                        for b in range(63):
                            nc.vector.tensor_scalar(
                                t[:], xt[:], float(kb[b]), float(cap[b]),
                                mybir.AluOpType.subtract, mybir.AluOpType.min,
                            )
                            nc.vector.tensor_scalar_max(t[:], t[:], 0.0)
                            nc.vector.scalar_tensor_tensor(
                                acc[:], t[:], tt[:, 1 + b:2 + b], acc[:],
                                mybir.AluOpType.mult, mybir.AluOpType.add,
                            )
                    elif mode == "clamp1":
                        nc.vector._custom_dve(
                            SPLINE_CLAMP1_INIT, out=acc[:], in0=xt[:],
                            s0=tt[:, 0:1], s1=tt[:, 1:2], imm2=float(cap[0]),
                        )
                        for b in range(1, 63):
                            nc.vector._custom_dve(
                                SPLINE_CLAMP1, out=acc[:], in0=xt[:], in1=acc[:],
                                s0=float(kb[b]), s1=tt[:, 1 + b:2 + b],
                                imm2=float(cap[b]),
                            )
                    elif mode == "relu2":
                        # y = 63 * x ; bin 0 handled exactly by INIT clamp,
                        # bins 1..62 as 31 relu pairs on y with unit spacing.
                        # tabs columns: 64 + b = g_b / 63 (b = 1..62)
                        yt = xp.tile([128, nf], F32, tag="y")
                        nc.vector._custom_dve(
                            SPLINE_CLAMP1_INIT, out=acc[:], in0=xt[:],
                            s0=tt[:, 0:1], s1=tt[:, 1:2], imm2=float(cap[0]),
                        )
                        nc.scalar.mul(yt[:], xt[:], 63.0)
                        for j in range(31):
                            b = 1 + 2 * j
                            nc.vector._custom_dve(
                                SPLINE_RELU2, out=acc[:], in0=yt[:], in1=acc[:],
                                s0=tt[:, 64 + b:65 + b], s1=tt[:, 65 + b:66 + b],
                                imm2=float(63.0 * kb[b]),
                            )
                    else:
                        raise ValueError(mode)
                    nc.sync.dma_start(
                        out_t[hf * 128:(hf + 1) * 128, pt * nf:(pt + 1) * nf], acc[:]
                    )

    nc.compile()
    return nc


# --------------------------------------------------------------------------- #
# Host wrapper
# --------------------------------------------------------------------------- #


def _make_tabs(knots: np.ndarray, coefs: np.ndarray):
    """Per-channel scalar tables + shared knot constants (float64 precompute)."""
    k64 = knots.astype(np.float64)
    c64 = coefs.astype(np.float64)
    h = np.diff(k64, axis=1)  # [M, 63]
    inv = 1.0 / (h + EPS)
    D = np.diff(c64, axis=1)  # [M, 63] saturated per-bin contribution

    tabs = np.zeros((M_CHANNELS, 256), dtype=np.float32)
    tabs[:, 0] = coefs[:, 0]
    tabs[:, 1:64] = (D * inv).astype(np.float32)
    # relu2 weights in y = 63x units: ramp slope w~_b = D_b/(63*h_b) (no eps:
    # saturated telescoping is then exact; only the active bin's slope is
    # off by eps/(h+eps), a non-cumulative ~6e-5 relative),
    # second difference g_1 = w~_1, g_b = w~_b - w~_{b-1}
    w = D / (h * 63.0)
    g = np.zeros((M_CHANNELS, 63), dtype=np.float64)
    g[:, 1] = w[:, 1]
    g[:, 2:] = w[:, 2:] - w[:, 1:-1]
    tabs[:, 64:127] = g.astype(np.float32)
    # cols 128+b: ACT relu bias = -63*k_b (for engine-split offload)
    tabs[:, 128:191] = np.broadcast_to(
        (-63.0 * k64[0, :63]).astype(np.float32)[None, :], (M_CHANNELS, 63)
    )

    kb = tuple(float(x) for x in k64[0])
    capb = tuple(float(x) for x in (h[0] + EPS))
    return tabs, kb, capb


def _make_in_map(u_t: np.ndarray, tabs: np.ndarray) -> dict:
    """Per-core input map for run_bass_kernel_spmd (hook for bench2)."""
    return {"u_t": u_t, "tabs": tabs}


def _knots_shared(knots: np.ndarray) -> bool:
    return bool((knots == knots[0:1]).all()) and knots[0, 0] == 0.0


def _reference_host(u, knots, coefs):
    """Numpy fallback (mirrors the reference op); only used if inputs ever
    break the shared-uniform-knots contract this kernel is specialized for."""
    m, K = knots.shape
    flat = u.reshape(-1, m).T
    idx = np.empty_like(flat, dtype=np.int64)
    for i in range(m):
        idx[i] = np.searchsorted(knots[i], flat[i], side="left")
    idx0 = np.clip(idx - 1, 0, K - 2)
    idx1 = idx0 + 1
    k0 = np.take_along_axis(knots, idx0, axis=1)
    k1 = np.take_along_axis(knots, idx1, axis=1)
    c0 = np.take_along_axis(coefs, idx0, axis=1)
    c1 = np.take_along_axis(coefs, idx1, axis=1)
    t = (flat - k0) / (k1 - k0 + EPS)
    out = c0 + t * (c1 - c0)
    return out.T.reshape(u.shape).astype(u.dtype)


def _run(u, knots, coefs, trace=False):
    u = np.asarray(u)
    knots = np.asarray(knots)
    coefs = np.asarray(coefs)
    orig_shape = u.shape
    if (
        u.ndim < 1
        or u.shape[-1] != M_CHANNELS
        or u.size != N_CORES * POINTS_PER_CORE * M_CHANNELS
        or knots.shape != (M_CHANNELS, N_KNOTS)
        or not _knots_shared(knots)
        or u.min() < 0.0
        or u.max() >= knots[0, -1] + 1e-12
    ):
        return _reference_host(u, knots, coefs), None

    tabs, kb, capb = _make_tabs(knots, coefs)
    nc = _build_module(MODE, POINTS_PER_CORE, NF, kb, capb)

    flat = np.ascontiguousarray(u.reshape(-1, M_CHANNELS))  # [262144, 256]
    shards = flat.reshape(N_CORES, POINTS_PER_CORE, M_CHANNELS)
    in_maps = []
    for c in range(N_CORES):
        u_t = np.ascontiguousarray(shards[c].T)  # [256, 32768]
        in_maps.append({"u_t": u_t, "tabs": tabs})

    res = run_bass_kernel_spmd(
        nc, in_maps, core_ids=list(range(N_CORES)), trace=trace
    )
    outs = [res.results[c]["out_t"].T for c in range(N_CORES)]  # [32768, 256] each
    out = np.concatenate(outs, axis=0).reshape(orig_shape).astype(np.float32)
    return out, res


def kernel(u: np.ndarray, knots: np.ndarray, coefs: np.ndarray) -> np.ndarray:
    out, _ = _run(u, knots, coefs, trace=False)
    return out

